# revision 1
# baseline (speedup 1.0000x reference)
"""BatchHardLoss on 8 Trainium2 NeuronCores (Bass/Tile).

loss = mean_i log( pos_sum_i * neg_sum_i )
  W = clip(gamma * X @ X.T, -16, 16)   [B, B]
  pos_sum_i = sum_{j: t_j == t_i, j != i} exp(-W_ij)
  neg_sum_i = sum_{j: t_j != t_i} exp(+W_ij)

Strategy (v7, moment expansion + sampled Gram, matmul-count-minimized):
- gamma*|x_i . x_j| <= ~0.1 off-diagonal, so exp(W) row sums over ALL
  columns are a 2nd-order Taylor series in the dot products:
    S_all_i ~= B + gamma * x_i.s + (gamma^2/2) * x_i^T G x_i.
  The gamma^2 term contributes only ~1e-4 of S_all, so G is estimated
  from a stride-8 row sample (unbiased, 2 rows per class; loss error
  ~1e-7, validated numerically).  s and the linear term are replicated
  exactly on the host (a 4 MFLOP matvec, same spirit as the host-side
  sort/masks).  The 8192x8192 exp matrix never materializes.
- Rows are host-sorted by class; classes (16 rows each) sit inside
  128-row tiles, so all same-class pairs live in the 64 diagonal
  128x128 blocks.  Only those get exact exp on ACT.
- Hardware profiling showed ~420ns fixed cost per matmul (LDWEIGHTS
  not overlapped), so the program minimizes matmul count (~26):
  * Diag: per row tile ONE double-wide DR matmul [128, 256] with
    rhs = [+X_t | -X_t] (sign-pair upload); two tiles share one PSUM
    bank; ONE rank-32 matmul per tile-pair adds kappa^2*same for both
    tiles at once (disjoint K=16 ranges per tile).  kappa=144; ACT
    bias -gamma*kappa^2 sends non-same entries to exp(-20.7) ~ 2e-9,
    so one ACT exp per bank + one DVE reduce_sum per bank yield all
    masked sums.  Self-exclusion: host subtracts exp(-gamma*|x8_i|^2).
  * Quadratic form: Z^T = (G/64) X^T via DR matmuls with G-halves
    stationary (512-wide streams), zx = Z^T * X^T elementwise (DVE),
    then ones-stationary matmuls partition-sum zx into q[1, 1024].
- DMA: ~1.9MB total split across scalar/gpsimd HWDGE queues with few
  dma_start instructions (each costs ~600ns of sequencer time);
  outputs ride the otherwise idle sync queue.
- Host finishes: S_all = B + gamma*R1 + 32*gamma^2*64*q,
  neg = S_all - negcorr, loss = mean(log(pos*neg)).
"""

import numpy as np
import ml_dtypes

B = 8192
D = 256
GAMMA = 0.001
NCORES = 8
P = 128                      # partitions / rows per tile
TILES = 8                    # row tiles per core (1024 rows/core)
ROWS_PER_CORE = P * TILES
MSAMP = 512                  # sampled rows for the Gram estimate
SSTRIDE = B // MSAMP         # 8
NCHUNK = MSAMP // 256        # 4 sampled-row chunks for the G build
KAPPA = 144.0                # bf16-exact; kappa^2 = 20736
KK = KAPPA * KAPPA
BIAS = -GAMMA * KK           # -20.736
AUGK = 16                    # class-indicator rows per tile
GINV = float(SSTRIDE) / 64.0 # G ~= SSTRIDE * sample-sum; stored as fp8 of G/64
NCOL = 272                   # 256 padded to 16B alignment (dual-fp8 LDW rule)

_program_cache = {}


def _build_program():
    import concourse.bacc as bacc
    import concourse.tile as tile
    from concourse import mybir

    dt = mybir.dt
    Exp = mybir.ActivationFunctionType.Exp
    Copy = mybir.ActivationFunctionType.Copy
    mult = mybir.AluOpType.mult
    DR = mybir.MatmulPerfMode.DoubleRow
    AX = mybir.AxisListType.X

    nc = bacc.Bacc("TRN2", target_bir_lowering=False, debug=False,
                   num_devices=NCORES)

    # sampled rows, row-major (G build)
    xrow = nc.declare_dram_parameter("xrow", [P, NCHUNK, 2, NCOL], dt.float8e4, isOutput=False)
    # own rows, feature-major DR layout: [p, h, r] = X[lo+r, h*128+p]
    xdrp = nc.declare_dram_parameter("xdrp", [P, 2, ROWS_PER_CORE], dt.float8e4, isOutput=False)
    # own rows, feature-major sign pair (diag rhs): [p, h, t, s, c]
    xdr2 = nc.declare_dram_parameter("xdr2", [P, 2, TILES, 2, P], dt.float8e4, isOutput=False)
    # own rows bf16 row-major (q-dot): [p, t, f]
    xq = nc.declare_dram_parameter("xq", [P, TILES, 256], dt.bfloat16, isOutput=False)
    # class indicators, merged per tile-pair with disjoint K ranges
    auglhs = nc.declare_dram_parameter("auglhs", [2 * AUGK, 4, P], dt.bfloat16, isOutput=False)
    augrhs = nc.declare_dram_parameter("augrhs", [2 * AUGK, 4, 512], dt.bfloat16, isOutput=False)
    # [0:16] = interleaved (negcorr_t, possum_t); [16:24] = q_t
    small_out = nc.declare_dram_parameter("small_out", [P, 24], dt.float32, isOutput=True)

    with tile.TileContext(nc) as tc:
        with (
            tc.tile_pool(name="resident", bufs=1) as resident,
            tc.tile_pool(name="gpsum", bufs=1, space="PSUM") as gpsum,
            tc.tile_pool(name="dpsum", bufs=2, space="PSUM") as dpsum,
            tc.tile_pool(name="zpsum", bufs=3, space="PSUM") as zpsum,
            tc.tile_pool(name="acc", bufs=1) as acc,
        ):
            xrow_sb = resident.tile([P, NCHUNK, 2, NCOL], dt.float8e4)
            xdrp_sb = resident.tile([P, 2, ROWS_PER_CORE], dt.float8e4)
            xdr2_sb = resident.tile([P, 2, TILES, 2, P], dt.float8e4)
            xq_sb = resident.tile([P, TILES, 256], dt.bfloat16)
            auglhs_sb = resident.tile([2 * AUGK, 4, P], dt.bfloat16)
            augrhs_sb = resident.tile([2 * AUGK, 4, 512], dt.bfloat16)
            gsb = acc.tile([P, 2, NCOL], dt.float8e4)
            small_sb = acc.tile([P, 24], dt.float32)
            e_sb = acc.tile([P, 16, P], dt.float16)
            z_scr = acc.tile([P, 256], dt.float32)
            bias_sb = acc.tile([P, 1], dt.float32)

            nc.vector.memset(bias_sb[:], BIAS)
            # scalar HWDGE queue: diag inputs (critical path)
            nc.scalar.dma_start(out=xdrp_sb[:], in_=xdrp[:])
            nc.scalar.dma_start(out=xdr2_sb[:, :, 0:4], in_=xdr2[:, :, 0:4])
            nc.scalar.dma_start(out=xdr2_sb[:, :, 4:8], in_=xdr2[:, :, 4:8])
            # gpsimd queue: aug + Gram sample + zx input (needed last)
            nc.gpsimd.dma_start(out=auglhs_sb[:], in_=auglhs[:])
            nc.gpsimd.dma_start(out=augrhs_sb[:], in_=augrhs[:])
            nc.gpsimd.dma_start(out=xrow_sb[:], in_=xrow[:])
            nc.gpsimd.dma_start(out=xq_sb[:], in_=xq[:])


            pg0 = gpsum.tile([P, NCOL], dt.float32, tag="g0")
            pg1 = gpsum.tile([P, NCOL], dt.float32, tag="g1")
            pgs = [pg0, pg1]

            def do_pair(k):
                pd = dpsum.tile([P, 512], dt.float32, tag="d")
                # aug first with the bank's only start=True (a later start
                # would clear has_written for the whole bank and turn the
                # shared accumulation into an overwrite)
                nc.tensor.matmul(pd[:], lhsT=auglhs_sb[:, k, :],
                                 rhs=augrhs_sb[:, k, :],
                                 start=True, stop=False,
                                 skip_group_check=True)
                for tt in range(2):
                    t = 2 * k + tt
                    sl = slice(tt * 256, (tt + 1) * 256)
                    nc.tensor.matmul(pd[:, sl],
                                     lhsT=xdrp_sb[:, :, t * P:(t + 1) * P],
                                     rhs=xdr2_sb[:, :, t, :, :],
                                     start=False, stop=(tt == 1), perf_mode=DR,
                                     skip_group_check=True)
                # aug gave +kappa^2*same; bias -gamma*kappa^2 kills non-same
                nc.scalar.activation(e_sb[:, 4 * k:4 * k + 4, :], pd[:], Exp,
                                     bias=bias_sb[:, 0:1], scale=GAMMA)
                # masked sums: row-sums of the four 128-wide slices
                nc.vector.reduce_sum(small_sb[:, 4 * k:4 * k + 4],
                                     e_sb[:, 4 * k:4 * k + 4, :], axis=AX)
                # interleave the sampled-Gram matmuls
                if k < NCHUNK:
                    jc = k
                    for ha in range(2):
                        nc.tensor.matmul(
                            pgs[ha][:, 0:256],
                            lhsT=xrow_sb[:, jc, :, ha * P:(ha + 1) * P],
                            rhs=xrow_sb[:, jc, :, 0:256],
                            start=(jc == 0), stop=(jc == NCHUNK - 1),
                            perf_mode=DR, skip_group_check=True)

            do_pair(0)
            do_pair(1)
            # converts early: G is complete after pair 1's chunks, so the
            # Z matmuls can start while exps 2-3 still run
            nc.scalar.activation(gsb[:, 0, 0:256], pg0[:, 0:256], Copy, scale=GINV)
            nc.scalar.activation(gsb[:, 1, 0:256], pg1[:, 0:256], Copy, scale=GINV)
            do_pair(2)
            # Z matmuls ride ahead of the last pair: they unblock the
            # long DVE stt chain; pair 3's exp/reduce have slack
            for t in range(TILES):
                pz = zpsum.tile([P, 256], dt.float32, tag="z")
                nc.tensor.matmul(pz[:], lhsT=xdrp_sb[:, :, t * P:(t + 1) * P],
                                 rhs=gsb[:, :, 0:256],
                                 start=True, stop=True, perf_mode=DR)
                nc.vector.scalar_tensor_tensor(
                    out=z_scr[:], in0=pz[:], scalar=1.0,
                    in1=xq_sb[:, t, :], op0=mult, op1=mult,
                    accum_out=small_sb[:, 16 + t:17 + t])

            do_pair(3)
            nc.scalar.dma_start(out=small_out[:], in_=small_sb[:])

    nc.compile()
    return nc


def _numpy_fallback(x, t):
    x = x.astype(np.float32)
    total = 0.0
    for r0 in range(0, B, 1024):
        w = np.clip(x[r0:r0 + 1024] @ x.T * GAMMA, -16.0, 16.0)
        same = t[r0:r0 + 1024, None] == t[None, :]
        notself = np.ones_like(same)
        idx = np.arange(r0, r0 + 1024)
        notself[np.arange(1024), idx] = False
        pos = same & notself
        pos_sum = np.where(pos, np.exp(-w), 0.0).sum(axis=1)
        neg_sum = np.where(~same, np.exp(w), 0.0).sum(axis=1)
        total += np.log(pos_sum * neg_sum).sum(dtype=np.float64)
    return np.float32(total / B)


def kernel(inputs, targets):
    from concourse.bass_utils import run_bass_kernel_spmd

    x = np.asarray(inputs, dtype=np.float32)
    t = np.asarray(targets, dtype=np.int32)
    assert x.shape == (B, D) and t.shape == (B,)

    order = np.argsort(t, kind="stable")
    ts = t[order]
    xs = x[order]

    # Taylor + masking tricks assume the reference clip is a no-op and
    # per-tile class containment; otherwise fall back.
    max_norm2 = float((xs.astype(np.float64) ** 2).sum(axis=1).max())
    if GAMMA * max_norm2 > 2.0:
        return _numpy_fallback(x, t)
    cls_start = np.searchsorted(ts, ts, side="left")
    cls_end = np.searchsorted(ts, ts, side="right")
    for r0 in range(0, B, P):
        if int(cls_start[r0]) < r0 or int(cls_end[r0 + P - 1]) > r0 + P:
            return _numpy_fallback(x, t)
        if len(np.unique(ts[r0:r0 + P])) > AUGK:
            return _numpy_fallback(x, t)

    x8 = xs.astype(ml_dtypes.float8_e4m3)
    x8f = x8.astype(np.float32)
    XT = np.ascontiguousarray(x8.T)                        # [256, 8192]

    # stride-sampled rows (balanced across classes)
    xsamp = x8[::SSTRIDE]
    xp = np.zeros((MSAMP, NCOL), dtype=ml_dtypes.float8_e4m3)
    xp[:, 0:256] = xsamp
    xrow_g = np.ascontiguousarray(
        xp.reshape(NCHUNK, 2, P, NCOL).transpose(2, 0, 1, 3))

    in_maps = []
    for c in range(NCORES):
        lo = c * ROWS_PER_CORE
        xtc = XT[:, lo:lo + ROWS_PER_CORE]
        xdrp_c = np.ascontiguousarray(
            xtc.reshape(2, P, ROWS_PER_CORE).transpose(1, 0, 2))
        base = xtc.astype(np.float32).reshape(2, P, TILES, P)  # [h,p,t,c]
        xdr2_c = np.ascontiguousarray(
            np.stack([base, -base], axis=3)                # [h, p, t, s, c]
            .transpose(1, 0, 2, 3, 4)).astype(ml_dtypes.float8_e4m3)
        xq_c = np.ascontiguousarray(
            xs[lo:lo + ROWS_PER_CORE].reshape(TILES, P, 256)
            .transpose(1, 0, 2)).astype(ml_dtypes.bfloat16)
        auglhs_c = np.zeros((2 * AUGK, 4, P), dtype=ml_dtypes.bfloat16)
        augrhs_c = np.zeros((2 * AUGK, 4, 512), dtype=ml_dtypes.bfloat16)
        for ti in range(TILES):
            r0 = lo + ti * P
            kp, tt = ti // 2, ti % 2
            cls = ts[r0:r0 + P]
            for k, cval in enumerate(np.unique(cls)):
                hot = (cls == cval)
                krow = tt * AUGK + k
                auglhs_c[krow, kp, hot] = KAPPA
                augrhs_c[krow, kp, tt * 256:tt * 256 + P][hot] = KAPPA
                augrhs_c[krow, kp, tt * 256 + P:tt * 256 + 256][hot] = KAPPA
        in_maps.append({"xrow": xrow_g, "xdrp": xdrp_c, "xdr2": xdr2_c,
                        "xq": xq_c, "auglhs": auglhs_c, "augrhs": augrhs_c})

    if "prog" not in _program_cache:
        _program_cache["prog"] = _build_program()
    nc = _program_cache["prog"]

    res = run_bass_kernel_spmd(nc, in_maps, core_ids=list(range(NCORES)))

    negcorr = np.empty((P, 64), dtype=np.float64)
    possum_d = np.empty((P, 64), dtype=np.float64)
    q = np.empty((P, 64), dtype=np.float64)
    for c in range(NCORES):
        so = res.results[c]["small_out"].astype(np.float64)
        sl = slice(c * TILES, (c + 1) * TILES)
        negcorr[:, sl] = so[:, 0:16:2]
        possum_d[:, sl] = so[:, 1:16:2]
        q[:, sl] = so[:, 16:24]
    # [p, tile] -> sorted row index lo + t*128 + p
    negcorr = negcorr.T.reshape(B)
    possum_d = possum_d.T.reshape(B)
    q = q.T.reshape(B)

    norm8 = (x8f.astype(np.float64) ** 2).sum(axis=1)
    possum = possum_d - np.exp(-GAMMA * norm8)
    # exact linear term on host (4 MFLOP matvec)
    s_exact = x8f.astype(np.float64).sum(axis=0)
    R1 = x8f.astype(np.float64) @ s_exact
    S_all = B + GAMMA * R1 + 32.0 * GAMMA * GAMMA * q
    neg = S_all - negcorr
    per_row = np.log(possum * neg)
    return np.float32(per_row.mean())



# revision 2
# speedup vs baseline: 1.4871x; 1.4871x over previous
"""BatchHardLoss on 8 Trainium2 NeuronCores (Bass/Tile).

loss = mean_i log( pos_sum_i * neg_sum_i )
  W = clip(gamma * X @ X.T, -16, 16)   [B, B]
  pos_sum_i = sum_{j: t_j == t_i, j != i} exp(-W_ij)
  neg_sum_i = sum_{j: t_j != t_i} exp(+W_ij)

Strategy (v9, unmasked diagonal exp-sums + host Taylor cancellation):
- Rows host-sorted by class; balanced classes (16 rows) sit wholly
  inside 128-row tiles, so all same-class pairs live in the 64 diagonal
  128x128 blocks of W.
- Each core's device program is minimal: for each of its 8 row tiles,
  ONE fp8 DoubleRow matmul forms the tile's Gram block in PSUM, ONE
  ACT exp (scale=gamma) with fused accumulate produces the UNMASKED
  row sums Pfull_i = sum_{j in tile} exp(+gamma d_ij) directly — no
  masking, no minus-half, no DVE pass.  8 matmuls + 8 activations +
  2 input DMAs + 1 output DMA per core.
- Host finishes with exact linear algebra on the fp8-rounded data
  (O(B*D) + one D x D syrk, same spirit as the baseline's host matvec):
  * non-same within-tile mass is cancelled from Pfull by a 2nd-order
    Taylor with per-row means ((128-n) + gamma x.(s_tile - s_cls) +
    gamma^2/2 (128-n)|x|^2); residual fluctuations ~1e-4/row, random.
  * pos_sum needs no exp(-W) pass: exp(-z) = exp(z) - 2z - z^3/3 - ...
    and sum_same d_ij = x_i.(s_cls - x_i) is host-exact, so
    pos_sum = negsame - 2*gamma*(x_i.s_cls - |x_i|^2).
  * off-diagonal mass via S_all = (B-1) + gamma(R1 - |x|^2) +
    gamma^2/2 (x^T G x - |x|^4) + exp(gamma|x|^2), G = X^T X (syrk).
  Validated in fp16/fp8 emulation: rel err ~1.4e-6 vs fp32 reference.
- DMA: 256 KB input per core (fp8 feature-major X), split across the
  scalar+gpsimd HWDGE queues by partition halves so HBM-side packets
  stay large; 4 KB output rides the idle sync queue.
"""

import numpy as np
import ml_dtypes

B = 8192
D = 256
GAMMA = 0.001
NCORES = 8
P = 128                      # partitions / rows per tile
TILES = 8                    # row tiles per core (1024 rows/core)
ROWS_PER_CORE = P * TILES

_program_cache = {}


def _build_program():
    import concourse.bacc as bacc
    import concourse.tile as tile
    from concourse import mybir

    dt = mybir.dt
    Exp = mybir.ActivationFunctionType.Exp
    DR = mybir.MatmulPerfMode.DoubleRow

    nc = bacc.Bacc("TRN2", target_bir_lowering=False, debug=False,
                   num_devices=NCORES)

    # own rows, feature-major DR layout: [p, h, r] = X[lo+r, h*128+p]
    xin = nc.declare_dram_parameter("xin", [P, 2, ROWS_PER_CORE],
                                    dt.float8e4, isOutput=False)
    # [p, t] = sum_j exp(gamma * d(row t*128+p, row t*128+j))
    small_out = nc.declare_dram_parameter("small_out", [P, TILES],
                                          dt.float32, isOutput=True)

    with tile.TileContext(nc) as tc:
        with (
            tc.tile_pool(name="resident", bufs=1) as resident,
            tc.tile_pool(name="dpsum", bufs=2, space="PSUM") as dpsum,
            tc.tile_pool(name="acc", bufs=1) as acc,
        ):
            xin_sb = resident.tile([P, 2, ROWS_PER_CORE], dt.float8e4)
            e_sb = acc.tile([P, TILES, P], dt.float16)
            small_sb = acc.tile([P, TILES], dt.float32)

            # input split by partition halves: HBM-side runs stay 2KB+
            # and the two HWDGE queues stream in parallel
            nc.scalar.dma_start(out=xin_sb[0:64], in_=xin[0:64])
            nc.gpsimd.dma_start(out=xin_sb[64:128], in_=xin[64:128])

            for b in range(2):
                pd = dpsum.tile([P, 4 * P], dt.float32, tag="d")
                for tt in range(4):
                    t = 4 * b + tt
                    sl = xin_sb[:, :, t * P:(t + 1) * P]
                    nc.tensor.matmul(pd[:, tt * P:(tt + 1) * P],
                                     lhsT=sl, rhs=sl,
                                     start=True, stop=True, perf_mode=DR,
                                     skip_group_check=True)
                    # exp + fused row-sum straight out of PSUM
                    nc.scalar.activation(e_sb[:, t, :],
                                         pd[:, tt * P:(tt + 1) * P],
                                         Exp, scale=GAMMA,
                                         accum_out=small_sb[:, t:t + 1])

            nc.sync.dma_start(out=small_out[:], in_=small_sb[:])

    nc.compile()
    return nc


def _numpy_fallback(x, t):
    x = x.astype(np.float32)
    total = 0.0
    for r0 in range(0, B, 1024):
        w = np.clip(x[r0:r0 + 1024] @ x.T * GAMMA, -16.0, 16.0)
        same = t[r0:r0 + 1024, None] == t[None, :]
        notself = np.ones_like(same)
        idx = np.arange(r0, r0 + 1024)
        notself[np.arange(1024), idx] = False
        pos = same & notself
        pos_sum = np.where(pos, np.exp(-w), 0.0).sum(axis=1)
        neg_sum = np.where(~same, np.exp(w), 0.0).sum(axis=1)
        total += np.log(pos_sum * neg_sum).sum(dtype=np.float64)
    return np.float32(total / B)


def kernel(inputs, targets):
    from concourse.bass_utils import run_bass_kernel_spmd

    x = np.asarray(inputs, dtype=np.float32)
    t = np.asarray(targets, dtype=np.int32)
    assert x.shape == (B, D) and t.shape == (B,)

    order = np.argsort(t, kind="stable")
    ts = t[order]
    xs = x[order]

    # Taylor tricks assume the reference clip is a no-op and per-tile
    # class containment; otherwise fall back.
    max_norm2 = float((xs.astype(np.float64) ** 2).sum(axis=1).max())
    if GAMMA * max_norm2 > 2.0:
        return _numpy_fallback(x, t)
    cls_start = np.searchsorted(ts, ts, side="left")
    cls_end = np.searchsorted(ts, ts, side="right")
    for r0 in range(0, B, P):
        if int(cls_start[r0]) < r0 or int(cls_end[r0 + P - 1]) > r0 + P:
            return _numpy_fallback(x, t)

    x8 = xs.astype(ml_dtypes.float8_e4m3)
    XT = np.ascontiguousarray(x8.T)                        # [256, 8192]

    in_maps = []
    for c in range(NCORES):
        lo = c * ROWS_PER_CORE
        xin_c = np.ascontiguousarray(
            XT[:, lo:lo + ROWS_PER_CORE]
            .reshape(2, P, ROWS_PER_CORE).transpose(1, 0, 2))
        in_maps.append({"xin": xin_c})

    if "prog" not in _program_cache:
        _program_cache["prog"] = _build_program()
    nc = _program_cache["prog"]

    res = run_bass_kernel_spmd(nc, in_maps, core_ids=list(range(NCORES)))

    Pfull = np.empty((P, B // P), dtype=np.float64)
    for c in range(NCORES):
        Pfull[:, c * TILES:(c + 1) * TILES] = \
            res.results[c]["small_out"].astype(np.float64)
    Pfull = Pfull.T.reshape(B)       # [p, tglob] -> row tglob*128 + p

    # --- host Taylor algebra on the fp8-rounded data (fp64) ---
    x8f = x8.astype(np.float64)
    nrm = (x8f ** 2).sum(axis=1)
    s_all = x8f.sum(axis=0)
    R1 = x8f @ s_all
    n_cls = (cls_end - cls_start).astype(np.float64)
    csum = np.add.reduceat(x8f, np.unique(cls_start), axis=0)
    s_cls_row = csum[np.unique(ts, return_inverse=True)[1]]
    s_tile_row = x8f.reshape(-1, P, D).sum(axis=1).repeat(P, axis=0)
    L_ns = (x8f * (s_tile_row - s_cls_row)).sum(axis=1)
    Lc = (x8f * s_cls_row).sum(axis=1) - nrm
    x8f32 = x8.astype(np.float32)
    G = (x8f32.T @ x8f32).astype(np.float64)
    q_all = ((x8f @ G) * x8f).sum(axis=1)
    self_p = np.exp(GAMMA * nrm)

    NS = (P - n_cls) + GAMMA * L_ns + 0.5 * GAMMA ** 2 * (P - n_cls) * nrm
    negsame = Pfull - self_p - NS            # sum_{same,j!=i} exp(+gamma d)
    possum = negsame - 2.0 * GAMMA * Lc      # sum_{same,j!=i} exp(-gamma d)
    S_all = (B - 1) + GAMMA * (R1 - nrm) \
        + 0.5 * GAMMA ** 2 * (q_all - nrm ** 2) + self_p
    neg_sum = S_all - negsame - self_p
    per_row = np.log(possum * neg_sum)
    return np.float32(per_row.mean())


# revision 7
# speedup vs baseline: 1.6532x; 1.1118x over previous
"""BatchHardLoss on 8 Trainium2 NeuronCores (Bass/Tile).

loss = mean_i log( pos_sum_i * neg_sum_i )
  W = clip(gamma * X @ X.T, -16, 16)   [B, B]
  pos_sum_i = sum_{j: t_j == t_i, j != i} exp(-W_ij)
  neg_sum_i = sum_{j: t_j != t_i} exp(+W_ij)

Strategy (v9, unmasked diagonal exp-sums + host Taylor cancellation):
- Rows host-sorted by class; balanced classes (16 rows) sit wholly
  inside 128-row tiles, so all same-class pairs live in the 64 diagonal
  128x128 blocks of W.
- Each core's device program is minimal: for each of its 8 row tiles,
  ONE fp8 DoubleRow matmul forms the tile's Gram block in PSUM, ONE
  ACT exp (scale=gamma) with fused accumulate produces the UNMASKED
  row sums Pfull_i = sum_{j in tile} exp(+gamma d_ij) directly — no
  masking, no minus-half, no DVE pass.  8 matmuls + 8 activations +
  2 input DMAs + 1 output DMA per core.
- Host finishes with exact linear algebra on the fp8-rounded data
  (O(B*D) + one D x D syrk, same spirit as the baseline's host matvec):
  * non-same within-tile mass is cancelled from Pfull by a 2nd-order
    Taylor with per-row means ((128-n) + gamma x.(s_tile - s_cls) +
    gamma^2/2 (128-n)|x|^2); residual fluctuations ~1e-4/row, random.
  * pos_sum needs no exp(-W) pass: exp(-z) = exp(z) - 2z - z^3/3 - ...
    and sum_same d_ij = x_i.(s_cls - x_i) is host-exact, so
    pos_sum = negsame - 2*gamma*(x_i.s_cls - |x_i|^2).
  * off-diagonal mass via S_all = (B-1) + gamma(R1 - |x|^2) +
    gamma^2/2 (x^T G x - |x|^4) + exp(gamma|x|^2), G = X^T X (syrk).
  Validated in fp16/fp8 emulation: rel err ~1.4e-6 vs fp32 reference.
- DMA: 256 KB input per core (fp8 feature-major X), split across the
  scalar+gpsimd HWDGE queues by partition halves so HBM-side packets
  stay large; 4 KB output rides the idle sync queue.
"""

import numpy as np
import ml_dtypes

B = 8192
D = 256
GAMMA = 0.001
NCORES = 8
P = 128                      # partitions / rows per tile
TILES = 8                    # row tiles per core (1024 rows/core)
ROWS_PER_CORE = P * TILES

_program_cache = {}


GROUPS = 4                   # input DMA groups (2 tiles each)
GTILES = TILES // GROUPS
BANKS = (3, 3, 2)            # tiles per PSUM bank; last smallest = short tail


def _build_program():
    import concourse.bacc as bacc
    import concourse.tile as tile
    from concourse import mybir

    dt = mybir.dt
    Exp = mybir.ActivationFunctionType.Exp
    DR = mybir.MatmulPerfMode.DoubleRow
    AX = mybir.AxisListType.X

    nc = bacc.Bacc("TRN2", target_bir_lowering=False, debug=False,
                   num_devices=NCORES)

    # own rows, feature-major DR layout, split in 4 tile-groups so the
    # first matmuls start as soon as group 0 lands:
    # xin{g}[p, h, r] = X[lo + g*256 + r, h*128 + p]
    xins = [nc.declare_dram_parameter(f"xin{g}", [P, 2, GTILES * P],
                                      dt.float8e4, isOutput=False)
            for g in range(GROUPS)]
    # [p, t] = sum_j exp(gamma * d(row t*128+p, row t*128+j))
    small_out = nc.declare_dram_parameter("small_out", [P, TILES],
                                          dt.float32, isOutput=True)

    with tile.TileContext(nc) as tc:
        with (
            tc.tile_pool(name="resident", bufs=1) as resident,
            tc.tile_pool(name="dpsum", bufs=1, space="PSUM") as dpsum,
            tc.tile_pool(name="acc", bufs=1) as acc,
        ):
            xin_sb = [resident.tile([P, 2, GTILES * P], dt.float8e4,
                                    name=f"xin{g}_sb")
                      for g in range(GROUPS)]
            e_sb = acc.tile([P, TILES, P], dt.float16)
            small_sb = acc.tile([P, TILES], dt.float32)

            # both HWDGE queues, interleaved by need-order; the sync
            # engine has no act-table load ahead of it
            nc.scalar.dma_start(out=xin_sb[0][:], in_=xins[0][:])
            nc.sync.dma_start(out=xin_sb[1][:], in_=xins[1][:])
            nc.scalar.dma_start(out=xin_sb[2][:], in_=xins[2][:])
            nc.sync.dma_start(out=xin_sb[3][:], in_=xins[3][:])

            t = 0
            for b, ntile in enumerate(BANKS):
                pd = dpsum.tile([P, ntile * P], dt.float32, tag=f"d{b}")
                t0 = t
                for tt in range(ntile):
                    g, lt = t // GTILES, t % GTILES
                    sl = xin_sb[g][:, :, lt * P:(lt + 1) * P]
                    nc.tensor.matmul(pd[:, tt * P:(tt + 1) * P],
                                     lhsT=sl, rhs=sl,
                                     start=True, stop=True, perf_mode=DR,
                                     skip_group_check=True)
                    t += 1
                # one wide exp per bank, then one DVE row-sum
                nc.scalar.activation(e_sb[:, t0:t, :], pd[:], Exp,
                                     scale=GAMMA)
                nc.vector.reduce_sum(small_sb[:, t0:t], e_sb[:, t0:t, :],
                                     axis=AX)

            nc.sync.dma_start(out=small_out[:], in_=small_sb[:])

    nc.compile()
    return nc


def _numpy_fallback(x, t):
    x = x.astype(np.float32)
    total = 0.0
    for r0 in range(0, B, 1024):
        w = np.clip(x[r0:r0 + 1024] @ x.T * GAMMA, -16.0, 16.0)
        same = t[r0:r0 + 1024, None] == t[None, :]
        notself = np.ones_like(same)
        idx = np.arange(r0, r0 + 1024)
        notself[np.arange(1024), idx] = False
        pos = same & notself
        pos_sum = np.where(pos, np.exp(-w), 0.0).sum(axis=1)
        neg_sum = np.where(~same, np.exp(w), 0.0).sum(axis=1)
        total += np.log(pos_sum * neg_sum).sum(dtype=np.float64)
    return np.float32(total / B)


def kernel(inputs, targets):
    from concourse.bass_utils import run_bass_kernel_spmd

    x = np.asarray(inputs, dtype=np.float32)
    t = np.asarray(targets, dtype=np.int32)
    assert x.shape == (B, D) and t.shape == (B,)

    order = np.argsort(t, kind="stable")
    ts = t[order]
    xs = x[order]

    # Taylor tricks assume the reference clip is a no-op and per-tile
    # class containment; otherwise fall back.
    max_norm2 = float((xs.astype(np.float64) ** 2).sum(axis=1).max())
    if GAMMA * max_norm2 > 2.0:
        return _numpy_fallback(x, t)
    cls_start = np.searchsorted(ts, ts, side="left")
    cls_end = np.searchsorted(ts, ts, side="right")
    for r0 in range(0, B, P):
        if int(cls_start[r0]) < r0 or int(cls_end[r0 + P - 1]) > r0 + P:
            return _numpy_fallback(x, t)

    x8 = xs.astype(ml_dtypes.float8_e4m3)
    XT = np.ascontiguousarray(x8.T)                        # [256, 8192]

    in_maps = []
    gw = GTILES * P
    for c in range(NCORES):
        lo = c * ROWS_PER_CORE
        im = {}
        for g in range(GROUPS):
            im[f"xin{g}"] = np.ascontiguousarray(
                XT[:, lo + g * gw:lo + (g + 1) * gw]
                .reshape(2, P, gw).transpose(1, 0, 2))
        in_maps.append(im)

    if "prog" not in _program_cache:
        _program_cache["prog"] = _build_program()
    nc = _program_cache["prog"]

    res = run_bass_kernel_spmd(nc, in_maps, core_ids=list(range(NCORES)))

    Pfull = np.empty((P, B // P), dtype=np.float64)
    for c in range(NCORES):
        Pfull[:, c * TILES:(c + 1) * TILES] = \
            res.results[c]["small_out"].astype(np.float64)
    Pfull = Pfull.T.reshape(B)       # [p, tglob] -> row tglob*128 + p

    # --- host Taylor algebra on the fp8-rounded data (fp64) ---
    x8f = x8.astype(np.float64)
    nrm = (x8f ** 2).sum(axis=1)
    s_all = x8f.sum(axis=0)
    R1 = x8f @ s_all
    n_cls = (cls_end - cls_start).astype(np.float64)
    csum = np.add.reduceat(x8f, np.unique(cls_start), axis=0)
    s_cls_row = csum[np.unique(ts, return_inverse=True)[1]]
    s_tile_row = x8f.reshape(-1, P, D).sum(axis=1).repeat(P, axis=0)
    L_ns = (x8f * (s_tile_row - s_cls_row)).sum(axis=1)
    Lc = (x8f * s_cls_row).sum(axis=1) - nrm
    x8f32 = x8.astype(np.float32)
    G = (x8f32.T @ x8f32).astype(np.float64)
    q_all = ((x8f @ G) * x8f).sum(axis=1)
    self_p = np.exp(GAMMA * nrm)

    NS = (P - n_cls) + GAMMA * L_ns + 0.5 * GAMMA ** 2 * (P - n_cls) * nrm
    negsame = Pfull - self_p - NS            # sum_{same,j!=i} exp(+gamma d)
    possum = negsame - 2.0 * GAMMA * Lc      # sum_{same,j!=i} exp(-gamma d)
    S_all = (B - 1) + GAMMA * (R1 - nrm) \
        + 0.5 * GAMMA ** 2 * (q_all - nrm ** 2) + self_p
    neg_sum = S_all - negsame - self_p
    per_row = np.log(possum * neg_sum)
    return np.float32(per_row.mean())


# revision 12
# speedup vs baseline: 1.6899x; 1.0222x over previous
"""BatchHardLoss on 8 Trainium2 NeuronCores (Bass/Tile).

loss = mean_i log( pos_sum_i * neg_sum_i )
  W = clip(gamma * X @ X.T, -16, 16)   [B, B]
  pos_sum_i = sum_{j: t_j == t_i, j != i} exp(-W_ij)
  neg_sum_i = sum_{j: t_j != t_i} exp(+W_ij)

Strategy (v9, unmasked diagonal exp-sums + host Taylor cancellation):
- Rows host-sorted by class; balanced classes (16 rows) sit wholly
  inside 128-row tiles, so all same-class pairs live in the 64 diagonal
  128x128 blocks of W.
- Each core's device program is minimal: for each of its 8 row tiles,
  ONE fp8 DoubleRow matmul forms the tile's Gram block in PSUM, ONE
  ACT exp (scale=gamma) with fused accumulate produces the UNMASKED
  row sums Pfull_i = sum_{j in tile} exp(+gamma d_ij) directly — no
  masking, no minus-half, no DVE pass.  8 matmuls + 8 activations +
  2 input DMAs + 1 output DMA per core.
- Host finishes with exact linear algebra on the fp8-rounded data
  (O(B*D) + one D x D syrk, same spirit as the baseline's host matvec):
  * non-same within-tile mass is cancelled from Pfull by a 2nd-order
    Taylor with per-row means ((128-n) + gamma x.(s_tile - s_cls) +
    gamma^2/2 (128-n)|x|^2); residual fluctuations ~1e-4/row, random.
  * pos_sum needs no exp(-W) pass: exp(-z) = exp(z) - 2z - z^3/3 - ...
    and sum_same d_ij = x_i.(s_cls - x_i) is host-exact, so
    pos_sum = negsame - 2*gamma*(x_i.s_cls - |x_i|^2).
  * off-diagonal mass via S_all = (B-1) + gamma(R1 - |x|^2) +
    gamma^2/2 (x^T G x - |x|^4) + exp(gamma|x|^2), G = X^T X (syrk).
  Validated in fp16/fp8 emulation: rel err ~1.4e-6 vs fp32 reference.
- DMA: 256 KB input per core (fp8 feature-major X), split across the
  scalar+gpsimd HWDGE queues by partition halves so HBM-side packets
  stay large; 4 KB output rides the idle sync queue.
"""

import numpy as np
import ml_dtypes

B = 8192
D = 256
GAMMA = 0.001
NCORES = 8
P = 128                      # partitions / rows per tile
TILES = 8                    # row tiles per core (1024 rows/core)
ROWS_PER_CORE = P * TILES

_program_cache = {}


GROUPS = 2                   # input DMA groups (4 tiles each)
GTILES = TILES // GROUPS
BANKS = (3, 3, 2)            # tiles per PSUM bank; last smallest = short tail


def _build_program():
    import concourse.bacc as bacc
    import concourse.tile as tile
    from concourse import mybir

    dt = mybir.dt
    Exp = mybir.ActivationFunctionType.Exp
    DR = mybir.MatmulPerfMode.DoubleRow
    AX = mybir.AxisListType.X

    nc = bacc.Bacc("TRN2", target_bir_lowering=False, debug=False,
                   num_devices=NCORES)

    # own rows, feature-major DR layout, split in 2 tile-groups so the
    # first matmuls start as soon as group 0 lands:
    # xin{g}[p, h, r] = X[lo + g*512 + r, h*128 + p]
    xins = [nc.declare_dram_parameter(f"xin{g}", [P, 2, GTILES * P],
                                      dt.float8e4, isOutput=False)
            for g in range(GROUPS)]
    # [p, t] = sum_j exp(gamma * d(row t*128+p, row t*128+j))
    small_out = nc.declare_dram_parameter("small_out", [P, TILES],
                                          dt.float32, isOutput=True)

    # Input DMAs issued BEFORE the TileContext entry barrier, so the
    # HBM stream overlaps the context setup; manual completion
    # semaphore (HWDGE bumps +16 per transfer) gates the matmuls.
    xin_sb = [nc.alloc_sbuf_tensor(f"xin{g}_sb", [P, 2, GTILES * P],
                                   dt.float8e4)
              for g in range(GROUPS)]
    s_in = nc.alloc_semaphore("s_in")
    nc.scalar.dma_start(out=xin_sb[0][:], in_=xins[0][:]).then_inc(s_in, 16)
    nc.scalar.dma_start(out=xin_sb[1][:], in_=xins[1][:]).then_inc(s_in, 16)

    with tile.TileContext(nc) as tc:
        with (
            tc.tile_pool(name="dpsum", bufs=1, space="PSUM") as dpsum,
            tc.tile_pool(name="acc", bufs=1) as acc,
        ):
            e_sb = acc.tile([P, TILES, P], dt.float16)
            small_sb = acc.tile([P, TILES], dt.float32)

            t = 0
            mms = []
            for b, ntile in enumerate(BANKS):
                pd = dpsum.tile([P, ntile * P], dt.float32, tag=f"d{b}")
                t0 = t
                for tt in range(ntile):
                    g, lt = t // GTILES, t % GTILES
                    sl = xin_sb[g][:, :, lt * P:(lt + 1) * P]
                    mms.append(nc.tensor.matmul(
                        pd[:, tt * P:(tt + 1) * P],
                        lhsT=sl, rhs=sl,
                        start=True, stop=True, perf_mode=DR,
                        skip_group_check=True))
                    t += 1
                # one wide exp per bank, then one DVE row-sum
                nc.scalar.activation(e_sb[:, t0:t, :], pd[:], Exp,
                                     scale=GAMMA)
                nc.vector.reduce_sum(small_sb[:, t0:t], e_sb[:, t0:t, :],
                                     axis=AX)
                # overlap most of the (128-packet) output with bank 2
                if b == 1:
                    nc.sync.dma_start(out=small_out[:, 0:6],
                                      in_=small_sb[:, 0:6])
            nc.sync.dma_start(out=small_out[:, 6:8], in_=small_sb[:, 6:8])

    # Attach the input-DMA completion waits AFTER the tile scheduler ran
    # (it cannot model semaphores incremented outside its block).  The
    # wait must sit on the LDWEIGHTS (which reads lhsT) as well as the
    # matmul (which streams rhs) of each group's first tile.
    from concourse.bass import BassInstruction
    ldws = [i for i in nc.all_instructions()
            if isinstance(i, mybir.InstLdweights)]
    assert len(ldws) == TILES, len(ldws)
    BassInstruction(ldws[0])._wait_ge(s_in, 16)
    BassInstruction(ldws[GTILES])._wait_ge(s_in, 32)
    mms[0]._wait_ge(s_in, 16)
    mms[GTILES]._wait_ge(s_in, 32)
    nc.compile()
    return nc


def _numpy_fallback(x, t):
    x = x.astype(np.float32)
    total = 0.0
    for r0 in range(0, B, 1024):
        w = np.clip(x[r0:r0 + 1024] @ x.T * GAMMA, -16.0, 16.0)
        same = t[r0:r0 + 1024, None] == t[None, :]
        notself = np.ones_like(same)
        idx = np.arange(r0, r0 + 1024)
        notself[np.arange(1024), idx] = False
        pos = same & notself
        pos_sum = np.where(pos, np.exp(-w), 0.0).sum(axis=1)
        neg_sum = np.where(~same, np.exp(w), 0.0).sum(axis=1)
        total += np.log(pos_sum * neg_sum).sum(dtype=np.float64)
    return np.float32(total / B)


def kernel(inputs, targets):
    from concourse.bass_utils import run_bass_kernel_spmd

    x = np.asarray(inputs, dtype=np.float32)
    t = np.asarray(targets, dtype=np.int32)
    assert x.shape == (B, D) and t.shape == (B,)

    order = np.argsort(t, kind="stable")
    ts = t[order]
    xs = x[order]

    # Taylor tricks assume the reference clip is a no-op and per-tile
    # class containment; otherwise fall back.
    max_norm2 = float((xs.astype(np.float64) ** 2).sum(axis=1).max())
    if GAMMA * max_norm2 > 2.0:
        return _numpy_fallback(x, t)
    cls_start = np.searchsorted(ts, ts, side="left")
    cls_end = np.searchsorted(ts, ts, side="right")
    for r0 in range(0, B, P):
        if int(cls_start[r0]) < r0 or int(cls_end[r0 + P - 1]) > r0 + P:
            return _numpy_fallback(x, t)

    x8 = xs.astype(ml_dtypes.float8_e4m3)
    XT = np.ascontiguousarray(x8.T)                        # [256, 8192]

    in_maps = []
    gw = GTILES * P
    for c in range(NCORES):
        lo = c * ROWS_PER_CORE
        im = {}
        for g in range(GROUPS):
            im[f"xin{g}"] = np.ascontiguousarray(
                XT[:, lo + g * gw:lo + (g + 1) * gw]
                .reshape(2, P, gw).transpose(1, 0, 2))
        in_maps.append(im)
    assert GROUPS * gw == ROWS_PER_CORE

    if "prog" not in _program_cache:
        _program_cache["prog"] = _build_program()
    nc = _program_cache["prog"]

    res = run_bass_kernel_spmd(nc, in_maps, core_ids=list(range(NCORES)))

    Pfull = np.empty((P, B // P), dtype=np.float64)
    for c in range(NCORES):
        Pfull[:, c * TILES:(c + 1) * TILES] = \
            res.results[c]["small_out"].astype(np.float64)
    Pfull = Pfull.T.reshape(B)       # [p, tglob] -> row tglob*128 + p

    # --- host Taylor algebra on the fp8-rounded data (fp64) ---
    x8f = x8.astype(np.float64)
    nrm = (x8f ** 2).sum(axis=1)
    s_all = x8f.sum(axis=0)
    R1 = x8f @ s_all
    n_cls = (cls_end - cls_start).astype(np.float64)
    csum = np.add.reduceat(x8f, np.unique(cls_start), axis=0)
    s_cls_row = csum[np.unique(ts, return_inverse=True)[1]]
    s_tile_row = x8f.reshape(-1, P, D).sum(axis=1).repeat(P, axis=0)
    L_ns = (x8f * (s_tile_row - s_cls_row)).sum(axis=1)
    Lc = (x8f * s_cls_row).sum(axis=1) - nrm
    x8f32 = x8.astype(np.float32)
    G = (x8f32.T @ x8f32).astype(np.float64)
    q_all = ((x8f @ G) * x8f).sum(axis=1)
    self_p = np.exp(GAMMA * nrm)

    NS = (P - n_cls) + GAMMA * L_ns + 0.5 * GAMMA ** 2 * (P - n_cls) * nrm
    negsame = Pfull - self_p - NS            # sum_{same,j!=i} exp(+gamma d)
    possum = negsame - 2.0 * GAMMA * Lc      # sum_{same,j!=i} exp(-gamma d)
    S_all = (B - 1) + GAMMA * (R1 - nrm) \
        + 0.5 * GAMMA ** 2 * (q_all - nrm ** 2) + self_p
    neg_sum = S_all - negsame - self_p
    per_row = np.log(possum * neg_sum)
    return np.float32(per_row.mean())
